# revision 19
# baseline (speedup 1.0000x reference)
"""Trainium2 Bass kernel for nn_CLsLoss (ABCD soft-region weighted histograms +
profile likelihood).

Strategy (data-parallel over events, 8 cores):
  - Each core gets 1/8 of the 4M bkg and 1/8 of the 4M sig events as
    [128, 3908] tiles (tail zero-weighted). f1/f2/w are fed as bf16 from the
    host; mt stays fp32 (bin-edge precision).
  - Radix-2 histogram: q = idx>>1 in bf16 via the +256 rint trick
    (qf = rint(mt*invw/2 + 127.5), sf = rint(mt*invw/2 + 128) so
    s = sf - qf = idx&1, all exact in bf16's [128,256) unit-ULP window).
  - Stationary per event-column: 24 is_equal(qf, 128+m) one-hot slabs plus a
    constant ones column (bin-pair 24 is recovered on host from the ones row).
  - Moving operand: 8 channels [w, w*s1, w*s2, w*s1*s2] and the same four
    gated by s (odd-parity partial sums). Even-parity bins are recovered on
    host as M0 - M1 (no even-gated products needed on device).
  - TensorE: psum[25, 8] += qoh[128,25]^T @ dsd[128,8] per column, one PSUM
    fp32 accumulation group per dataset.
  - Host: sum per-core [25,16] partials in float64, unmix parity + ones row,
    derive regions A/B/C/D, evaluate the [50]-bin profile likelihood.
  - Runtime params (cuts, bin affine coeffs) ride in 8 leading columns of the
    mt_bkg tensor (saves a DMA issue slot at pipeline fill); chunk schedules
    ramp in/out (326/978) and DMA emission is mt-first since qf gates the
    one-hot slabs.
"""

import os as _os

import numpy as np

NBIN = 50
N_EVENTS = 4_000_000
NCORES = 8
NPC = N_EVENTS // NCORES          # 500_000 events per core per dataset
P = 128
COLS = 3908                       # 128*3908 = 500224 >= NPC (tail zero-weighted)
QP = 25                           # bin pairs (q = idx >> 1)
NCH = 4                           # weight channels: 1, s1, s2, s1*s2
INT_LUMI = 117100.0
EPS = 1e-6
STEEPNESS = 20.0

CHUNK = int(_os.environ.get("K_CHUNK", "1303"))
QOH_GP = _os.environ.get("K_QOH_GP", "2")          # qoh slabs on GpSimd (cycle)
RAMP = _os.environ.get("K_RAMP", "326,978")          # leading warmup chunks
RAMPOUT = _os.environ.get("K_RAMPOUT", "978,326")    # trailing drain chunks
S_ENG = _os.environ.get("K_S_ENG", "vector")
ONES_MODE = _os.environ.get("K_ONES", "act")
WARM = int(_os.environ.get("K_WARM", "1"))
EARLYCP = int(_os.environ.get("K_EARLYCP", "0"))
SMALLGP = int(_os.environ.get("K_SMALLGP", "7"))
GPFRAC = float(_os.environ.get("K_GPFRAC", "0.0"))
DMA_Q2 = _os.environ.get("K_DMA_Q2", "")             # 2nd DMA issue queue
CP_ENG = _os.environ.get("K_CP_ENG", "vector")       # psum->sbuf copy engine
SPLIT_OUT = int(_os.environ.get("K_SPLIT_OUT", "0"))
GCHAIN = _os.environ.get("K_GCHAIN", "chain")
MMSTEP = int(_os.environ.get("K_MMSTEP", "1"))       # diagnostic only


def _build_program():
    import concourse.bass as bass
    import concourse.bacc as bacc
    import concourse.mybir as mybir
    import concourse.tile as tile

    dt = mybir.dt
    Alu = mybir.AluOpType
    Act = mybir.ActivationFunctionType

    nc = bacc.Bacc("TRN2", target_bir_lowering=False, debug=False,
                   num_devices=NCORES)

    din = {}
    for ds in ("bkg", "sig"):
        din[f"f1_{ds}"] = nc.dram_tensor(f"f1_{ds}", [P, COLS], dt.bfloat16,
                                         kind="ExternalInput")
        din[f"f2_{ds}"] = nc.dram_tensor(f"f2_{ds}", [P, COLS], dt.bfloat16,
                                         kind="ExternalInput")
        mt_cols = COLS + 8 if ds == "bkg" else COLS
        din[f"mt_{ds}"] = nc.dram_tensor(f"mt_{ds}", [P, mt_cols], dt.float32,
                                         kind="ExternalInput")
        din[f"w_{ds}"] = nc.dram_tensor(f"w_{ds}", [P, COLS], dt.bfloat16,
                                        kind="ExternalInput")
    dout = nc.dram_tensor("hist_out", [QP, 4 * NCH], dt.float32,
                          kind="ExternalOutput")

    def make_chunks(ramp, rampout):
        head, c0 = [], 0
        for r in ramp:
            head.append((c0, r))
            c0 += r
        tail_widths = list(rampout)
        c1 = COLS - sum(tail_widths)
        mid, cm = [], c0
        while cm < c1:
            mid.append((cm, min(CHUNK, c1 - cm)))
            cm += CHUNK
        tail = []
        for r in tail_widths:
            tail.append((c1, r))
            c1 += r
        return head + mid + tail

    ramp = [int(x) for x in RAMP.split(",") if x]
    rampout = [int(x) for x in RAMPOUT.split(",") if x]
    chunk_sets = {"bkg": make_chunks(ramp, []),
                  "sig": make_chunks([], rampout)}
    qoh_gp_cycle = [int(x) for x in QOH_GP.split(",") if x]

    from contextlib import ExitStack
    with tile.TileContext(nc) as tc, ExitStack() as ctx:
        io_pool = ctx.enter_context(tc.tile_pool(name="io", bufs=2))
        mid_pool = ctx.enter_context(tc.tile_pool(name="mid", bufs=2))
        qoh_pool = ctx.enter_context(tc.tile_pool(name="qoh", bufs=2))
        const_pool = ctx.enter_context(tc.tile_pool(name="const", bufs=1))
        psum_pool = ctx.enter_context(
            tc.tile_pool(name="psum", bufs=1, space=bass.MemorySpace.PSUM))
        out_pool = ctx.enter_context(tc.tile_pool(name="out", bufs=1))

        if WARM:
            warm = const_pool.tile([P, 8], dt.bfloat16)
            nc.vector.memset(warm[:], 0.0)
            nc.scalar.activation(warm[:], warm[:], Act.Sigmoid)

        par = const_pool.tile([P, 8], dt.float32)
        bias1 = par[:, 0:1]      # -20*cut1
        bias2 = par[:, 1:2]      # -20*cut2
        hinvw = par[:, 2:3]      # 0.5/bin_width
        qbias = par[:, 3:4]      # 127.5 - e0/(2w)
        sbias = par[:, 4:5]      # 128.0 - e0/(2w)

        psums = {ds: psum_pool.tile([QP, 2 * NCH], dt.float32,
                                    name=f"ps_{ds}", tag=f"ps_{ds}")
                 for ds in ("bkg", "sig")}

        out_sb = out_pool.tile([QP, 4 * NCH], dt.float32)
        for dsi, ds in enumerate(("bkg", "sig")):
            ps = psums[ds]
            chunks = chunk_sets[ds]
            gi = 0
            for ci, (c0, cw) in enumerate(chunks):
                f1 = io_pool.tile([P, cw], dt.bfloat16, tag="f1", bufs=1)
                f2 = io_pool.tile([P, cw], dt.bfloat16, tag="f2")
                # dsd channels: w, w*s1, w*s2, w*s1*s2, then the same with
                # w replaced by ws = w*s (parity-gated chain)
                dsd = mid_pool.tile([P, 8 * cw], dt.bfloat16, tag="dsd")
                dq2 = getattr(nc, DMA_Q2) if DMA_Q2 else nc.sync
                if ds == "bkg" and ci == 0:
                    # params ride in the leading 8 columns of mt_bkg
                    mt0 = io_pool.tile([P, 8 + cw], dt.float32, tag="mt")
                    nc.sync.dma_start(mt0[:], din["mt_bkg"][:, 0:8 + cw])
                    nc.vector.tensor_copy(par[:], mt0[:, 0:8])
                    mt_ap = mt0[:, 8:8 + cw]
                else:
                    off = 8 if ds == "bkg" else 0
                    mt_t = io_pool.tile([P, cw], dt.float32, tag="mt")
                    nc.sync.dma_start(mt_t[:],
                                      din[f"mt_{ds}"][:, off + c0:off + c0 + cw])
                    mt_ap = mt_t[:]
                nc.sync.dma_start(dsd[:, 0:cw], din[f"w_{ds}"][:, c0:c0 + cw])
                dq2.dma_start(f1[:], din[f"f1_{ds}"][:, c0:c0 + cw])
                dq2.dma_start(f2[:], din[f"f2_{ds}"][:, c0:c0 + cw])

                s12 = mid_pool.tile([P, 2 * cw], dt.bfloat16, tag="s12")
                qf = mid_pool.tile([P, cw], dt.bfloat16, tag="qf")
                sf = mid_pool.tile([P, cw], dt.bfloat16, tag="sf", bufs=1)
                sb = mid_pool.tile([P, cw], dt.bfloat16, tag="sb", bufs=1)
                nc.scalar.activation(qf[:], mt_ap,
                                     Act.Identity, bias=qbias, scale=hinvw)
                nc.scalar.activation(sf[:], mt_ap,
                                     Act.Identity, bias=sbias, scale=hinvw)
                nc.scalar.activation(s12[:, 0:cw], f1[:], Act.Sigmoid,
                                     bias=bias1, scale=STEEPNESS)
                nc.scalar.activation(s12[:, cw:2 * cw], f2[:], Act.Sigmoid,
                                     bias=bias2, scale=STEEPNESS)
                nc.vector.tensor_tensor(sb[:], sf[:], qf[:], Alu.subtract)

                # ungated chain on GpSimd: [d1|d2] = w*(s1|s2), d12 = d1*s2
                w_b = dsd[:, 0:cw].rearrange("p (o t) -> p o t", o=1)
                w_b = w_b.to_broadcast((P, 2, cw))
                s12_r = s12[:].rearrange("p (r t) -> p r t", r=2)
                nc.gpsimd.tensor_tensor(
                    dsd[:, cw:3 * cw].rearrange("p (r t) -> p r t", r=2),
                    w_b, s12_r, Alu.mult)
                nc.gpsimd.tensor_tensor(dsd[:, 3 * cw:4 * cw],
                                        dsd[:, cw:2 * cw],
                                        s12[:, cw:2 * cw], Alu.mult)

                if GCHAIN == "bcast4":
                    # gated chain in one op: dsd[4:8] = s * dsd[0:4]
                    # (GP's d12 lands ~4us before DVE consumes it here)
                    d_b = dsd[:, 0:4 * cw].rearrange("p (r t) -> p r t",
                                                     r=NCH)
                    s_b = sb[:].rearrange("p (o t) -> p o t", o=1)
                    s_b = s_b.to_broadcast((P, NCH, cw))
                    nc.vector.tensor_tensor(
                        dsd[:, 4 * cw:8 * cw].rearrange("p (r t) -> p r t",
                                                        r=NCH),
                        s_b, d_b, Alu.mult)
                else:
                    # gated chain on DVE: ws = w*s, [g1|g2] = ws*(s1|s2),
                    # g12 = g1*s2
                    nc.vector.tensor_tensor(dsd[:, 4 * cw:5 * cw],
                                            dsd[:, 0:cw], sb[:], Alu.mult)
                    ws_b = dsd[:, 4 * cw:5 * cw].rearrange(
                        "p (o t) -> p o t", o=1)
                    ws_b = ws_b.to_broadcast((P, 2, cw))
                    nc.vector.tensor_tensor(
                        dsd[:, 5 * cw:7 * cw].rearrange("p (r t) -> p r t",
                                                        r=2),
                        ws_b, s12_r, Alu.mult)
                    nc.vector.tensor_tensor(dsd[:, 7 * cw:8 * cw],
                                            dsd[:, 5 * cw:6 * cw],
                                            s12[:, cw:2 * cw], Alu.mult)

                # one-hot slabs over bin pairs + constant ones column
                qoh = qoh_pool.tile([P, QP * cw], dt.bfloat16, tag="qoh")
                n_gp = qoh_gp_cycle[gi % len(qoh_gp_cycle)]
                gi += 1
                if cw < 978:
                    n_gp = SMALLGP
                split = GPFRAC if (n_gp < QP - 1 and cw >= 978) else 0.0
                for m in range(QP - 1):
                    lo = m * cw
                    if m == n_gp and split > 0.0:
                        h = int(cw * (1.0 - split))
                        nc.vector.tensor_scalar(qoh[:, lo:lo + h], qf[:, 0:h],
                                                128.0 + m, None, Alu.is_equal)
                        nc.gpsimd.tensor_scalar(qoh[:, lo + h:lo + cw],
                                                qf[:, h:cw],
                                                128.0 + m, None, Alu.is_equal)
                        continue
                    eng = nc.gpsimd if m < n_gp else nc.vector
                    eng.tensor_scalar(qoh[:, lo:lo + cw], qf[:],
                                      128.0 + m, None, Alu.is_equal)
                if ONES_MODE == "act":
                    nc.scalar.activation(qoh[:, (QP - 1) * cw:QP * cw], qf[:],
                                         Act.Identity, bias=1.0, scale=0.0)
                else:
                    nc.vector.memset(qoh[:, (QP - 1) * cw:QP * cw], 1.0)

                qoh_r = qoh[:].rearrange("p (m t) -> p t m", t=cw)
                dsd_r = dsd[:].rearrange("p (j t) -> p t j", t=cw)
                last_chunk = ci == len(chunks) - 1
                for t in range(0, cw, MMSTEP):
                    nc.tensor.matmul(ps[:], qoh_r[:, t, :], dsd_r[:, t, :],
                                     start=(ci == 0 and t == 0),
                                     stop=(last_chunk and t >= cw - MMSTEP),
                                     skip_group_check=True)
            def _cp(dst, src_ps):
                if CP_ENG == "scalar":
                    nc.scalar.activation(dst, src_ps, Act.Identity)
                else:
                    nc.vector.tensor_copy(dst, src_ps)

            if EARLYCP or ds == "sig":
                _cp(out_sb[:, dsi * 2 * NCH:(dsi + 1) * 2 * NCH], ps[:])
            if not EARLYCP and ds == "sig":
                _cp(out_sb[:, 0:2 * NCH], psums["bkg"][:])
            if SPLIT_OUT and ds == "bkg" and EARLYCP:
                nc.sync.dma_start(dout[:, 0:2 * NCH], out_sb[:, 0:2 * NCH])

        if SPLIT_OUT and EARLYCP:
            nc.sync.dma_start(dout[:, 2 * NCH:4 * NCH],
                              out_sb[:, 2 * NCH:4 * NCH])
        else:
            nc.sync.dma_start(dout[:], out_sb[:])

    nc.compile()
    return nc


def _shard(arr: np.ndarray, core: int, dtype,
           prepend: np.ndarray | None = None) -> np.ndarray:
    sl = arr[core * NPC:(core + 1) * NPC]
    out = np.zeros(P * COLS, dtype=np.float32)
    out[:NPC] = sl
    out = out.reshape(P, COLS).astype(dtype)
    if prepend is not None:
        out = np.concatenate([prepend.astype(dtype), out], axis=1)
    return out


def _unmix(h: np.ndarray) -> np.ndarray:
    """h: [25, 8] psum block -> [NBIN, 4] channel hist (H,H1,H2,H12)."""
    M0 = h[:, 0:NCH].copy()          # pair sums (rows 0..23), ones row at 24
    M1 = h[:, NCH:2 * NCH].copy()    # odd-parity partial sums
    M0[QP - 1] = h[QP - 1, 0:NCH] - M0[:QP - 1].sum(axis=0)
    M1[QP - 1] = h[QP - 1, NCH:2 * NCH] - M1[:QP - 1].sum(axis=0)
    out = np.empty((NBIN, NCH), dtype=np.float64)
    out[0::2] = M0 - M1              # even bins
    out[1::2] = M1                   # odd bins
    return out


def _regions(full: np.ndarray) -> np.ndarray:
    """[NBIN, 4] channels (H, H1, H2, H12) -> regions (A,B,C,D) * INT_LUMI."""
    H, H1, H2, H12 = full[:, 0], full[:, 1], full[:, 2], full[:, 3]
    A = H1 - H12
    B = H12
    C = H - H1 - H2 + H12
    D = H2 - H12
    return np.stack([A, B, C, D], axis=-1) * INT_LUMI


def _likelihood(hb: np.ndarray, hs: np.ndarray) -> float:
    """hb/hs: [NBIN, 4] region histograms (A,B,C,D) in float64."""
    from scipy.special import gammaln

    obs_A, obs_B, obs_C, obs_D = hb[:, 0], hb[:, 1], hb[:, 2], hb[:, 3]
    S_A, S_B, S_C, S_D = hs[:, 0], hs[:, 1], hs[:, 2], hs[:, 3]
    mu = 1.0
    # theta = 0, nA/nC/nD = obs_A/obs_C/obs_D
    exp_A = obs_A + mu * S_A
    exp_C = obs_C + mu * S_C
    exp_D = obs_D + mu * S_D
    bkg_SR = obs_A * obs_D / (obs_C + EPS)
    exp_B = bkg_SR + mu * S_B

    def pois(o, e):
        return o * np.log(e + EPS) - e - gammaln(o + 1.0)

    llh = (pois(obs_A, exp_A) + pois(obs_B, exp_B)
           + pois(obs_C, exp_C) + pois(obs_D, exp_D))
    return -float(llh.sum())


_NC_CACHE = None
LAST_RESULTS = None


def kernel(f1_bkg, f2_bkg, mt_bkg, w_bkg, f1_sig, f2_sig, mt_sig, w_sig,
           cut1, cut2, mt_bin_edges):
    global _NC_CACHE, LAST_RESULTS
    import ml_dtypes
    from concourse.bass_utils import run_bass_kernel_spmd

    if _NC_CACHE is None:
        _NC_CACHE = _build_program()
    nc = _NC_CACHE

    bf16 = ml_dtypes.bfloat16
    edges = np.asarray(mt_bin_edges, dtype=np.float64)
    width = float(edges[1] - edges[0])
    e0 = float(edges[0])
    hw = 0.5 / width
    par = np.zeros((P, 8), dtype=np.float32)
    par[:, 0] = -STEEPNESS * float(cut1)
    par[:, 1] = -STEEPNESS * float(cut2)
    par[:, 2] = hw
    par[:, 3] = 127.5 - e0 * hw
    par[:, 4] = 128.0 - e0 * hw

    arrs = {
        "f1_bkg": (f1_bkg, bf16), "f2_bkg": (f2_bkg, bf16),
        "mt_bkg": (mt_bkg, np.float32), "w_bkg": (w_bkg, bf16),
        "f1_sig": (f1_sig, bf16), "f2_sig": (f2_sig, bf16),
        "mt_sig": (mt_sig, np.float32), "w_sig": (w_sig, bf16),
    }
    arrs = {k: (np.asarray(v, dtype=np.float32), t) for k, (v, t) in
            arrs.items()}

    in_maps = []
    for core in range(NCORES):
        m = {k: _shard(v, core, t, prepend=par if k == "mt_bkg" else None)
             for k, (v, t) in arrs.items()}
        in_maps.append(m)

    try:
        res = run_bass_kernel_spmd(nc, in_maps, core_ids=list(range(NCORES)))
    except Exception:
        # transient device states typically clear on retry
        res = run_bass_kernel_spmd(nc, in_maps, core_ids=list(range(NCORES)))
    LAST_RESULTS = res

    total = np.zeros((QP, 4 * NCH), dtype=np.float64)
    for rmap in res.results:
        total += rmap["hist_out"].astype(np.float64)

    hb = _regions(_unmix(total[:, 0:2 * NCH]))
    hs = _regions(_unmix(total[:, 2 * NCH:4 * NCH]))
    out = _likelihood(hb, hs)
    return np.float32(out)
